# revision 14
# baseline (speedup 1.0000x reference)
"""KmeansQuantizer Bass kernel for Trainium2, data-parallel over 8 NeuronCores.

Reference computation (N=131072, D=128, K=1024):
    distances[n,k] = ||x_n||^2 - 2 x_n.c_k + ||c_k||^2        [N, K] f32
    labels[n]     = argmin_k distances[n,k]                    [N]    i32
    preds[n,:]    = codebook[labels[n]]                        [N, D] f32

Sharding: x split along N across 8 cores (16384 rows each), codebook
replicated.  Per 128-row tile on each core:
    PE   : transpose x-tile; cross = xT.T @ (2*C^T) into PSUM (fp32)
    DVE  : neg = cross - csq  (tensor_tensor_reduce, fused row-max)
           max_index -> argmax(neg) == argmin(dist)
    ACT  : x_sq (Square+accum), dist = -neg + x_sq  (scale/bias)
    GpSimd: indirect gather codebook[label] -> preds tile
    DMA  : contiguous 512KB dist tiles, 64KB x/preds tiles
"""

import os
import numpy as np

NO_GATHER = os.environ.get("KQ_NO_GATHER") == "1"
NO_LABELS = os.environ.get("KQ_NO_LABELS") == "1"
NO_TTR = os.environ.get("KQ_TTR") != "1"  # plain TT+max path is default (HW-verified)
NO_ARGMIN = os.environ.get("KQ_NO_ARGMIN") == "1"
NO_XSQ = os.environ.get("KQ_NO_XSQ") == "1"

N, D, K = 131072, 128, 1024
N_CORES = 8
N_LOCAL = N // N_CORES          # 16384
P = 128                         # partitions / tile rows
N_TILES_FULL = N_LOCAL // P     # 128


def build(n_tiles=N_TILES_FULL):
    import concourse.bass as bass
    import concourse.mybir as mybir
    from concourse import bacc
    from concourse.bass import ts
    from concourse.masks import make_identity
    from concourse.tile import TileContext

    f32 = mybir.dt.float32
    u32 = mybir.dt.uint32
    i32 = mybir.dt.int32

    n_local = n_tiles * P
    nc = bacc.Bacc("TRN2", target_bir_lowering=False, debug=False)

    x_dram = nc.dram_tensor("x", [n_local, D], f32, kind="ExternalInput").ap()
    cb_dram = nc.dram_tensor("codebook", [K, D], f32, kind="ExternalInput").ap()
    dist_dram = nc.dram_tensor("dist", [n_local, K], f32, kind="ExternalOutput").ap()
    preds_dram = nc.dram_tensor("preds", [n_local, D], f32, kind="ExternalOutput").ap()
    labels_dram = nc.dram_tensor("labels", [n_local], i32, kind="ExternalOutput").ap()

    labels2d = labels_dram.rearrange("(t p) -> t p", p=P)

    with TileContext(nc) as tc:
        with (
            tc.tile_pool(name="const", bufs=1) as constp,
            tc.tile_pool(name="xin", bufs=4) as xinp,
            tc.tile_pool(name="xtsb", bufs=3) as xtp,
            tc.tile_pool(name="small", bufs=4) as smallp,
            tc.tile_pool(name="neg", bufs=3) as negp,
            tc.tile_pool(name="dist", bufs=4) as distp,
            tc.tile_pool(name="predst", bufs=4) as predsp,
            tc.tile_pool(name="ps_t", bufs=2, space="PSUM") as ps_t,
            tc.tile_pool(name="ps_x", bufs=2, space="PSUM") as ps_x,
        ):
            # ---------------- setup: identity, C^T, 2C^T, csq broadcast ----
            identity = constp.tile([P, P], f32)
            make_identity(nc, identity)
            ones = constp.tile([P, P], f32)
            nc.gpsimd.memset(ones, 1.0)

            ct2 = constp.tile([P, K], f32)     # 2 * C^T  [d, k]
            ctsq = constp.tile([P, K], f32)    # (C^T)^2  [d, k]
            for kb in range(K // P):
                cbt = xinp.tile([P, P], f32, tag="cbload")
                nc.sync.dma_start(out=cbt, in_=cb_dram[ts(kb, P), :])
                pt = ps_t.tile([P, P], f32, tag="tp")
                nc.tensor.transpose(out=pt, in_=cbt, identity=identity)
                nc.scalar.mul(ct2[:, ts(kb, P)], pt, 2.0)
                nc.scalar.square(ctsq[:, ts(kb, P)], pt)

            # csq[k] broadcast to all partitions via ones-matmul
            csq_ps = ps_x.tile([P, K], f32, tag="cross")
            for h in range(K // 512):
                nc.tensor.matmul(
                    out=csq_ps[:, ts(h, 512)], lhsT=ones, rhs=ctsq[:, ts(h, 512)],
                    start=True, stop=True,
                )
            csq = constp.tile([P, K], f32)
            nc.scalar.copy(csq, csq_ps)


            # ---------------- main loop over 128-row tiles -----------------
            for t in range(n_tiles):
                x_t = xinp.tile([P, D], f32)
                nc.sync.dma_start(out=x_t, in_=x_dram[ts(t, P), :])

                sq_scr = smallp.tile([P, D], f32, tag="sqscr")
                x_sq = smallp.tile([P, 1], f32, tag="xsq")
                if NO_XSQ:
                    nc.vector.memset(x_sq, 0.0)
                else:
                    nc.scalar.activation(sq_scr, x_t,
                                         mybir.ActivationFunctionType.Square,
                                         accum_out=x_sq)

                xT_ps = ps_t.tile([P, P], f32, tag="tp")
                nc.tensor.transpose(out=xT_ps, in_=x_t, identity=identity)
                xT = xtp.tile([P, P], f32)
                nc.scalar.copy(xT, xT_ps)

                cross_ps = ps_x.tile([P, K], f32, tag="cross")
                for h in range(K // 512):
                    nc.tensor.matmul(
                        out=cross_ps[:, ts(h, 512)], lhsT=xT,
                        rhs=ct2[:, ts(h, 512)], start=True, stop=True,
                    )

                # neg = 2*cross - csq ; rowmax(neg) fused into same DVE pass
                neg = negp.tile([P, K], f32)
                maxv = smallp.tile([P, 8], f32, tag="maxv")
                idx8 = smallp.tile([P, 8], u32, tag="idx8")
                if NO_ARGMIN:
                    nc.vector.tensor_tensor(out=neg, in0=cross_ps, in1=csq,
                                            op=mybir.AluOpType.subtract)
                    nc.vector.memset(idx8, 0)
                elif NO_TTR:
                    nc.gpsimd.memset(maxv, -3.0e38)
                    nc.vector.tensor_tensor(out=neg, in0=cross_ps, in1=csq,
                                            op=mybir.AluOpType.subtract)
                    nc.vector.max(out=maxv, in_=neg)
                    nc.vector.max_index(out=idx8, in_max=maxv, in_values=neg)
                else:
                    nc.vector.tensor_tensor_reduce(
                        out=neg, in0=cross_ps, in1=csq, scale=1.0, scalar=-3.0e38,
                        op0=mybir.AluOpType.subtract, op1=mybir.AluOpType.max,
                        accum_out=maxv[:, 0:1],
                    )
                    nc.gpsimd.memset(maxv[:, 1:8], -3.0e38)
                    nc.vector.max_index(out=idx8, in_max=maxv, in_values=neg)

                # dist = -neg + x_sq
                dist_sb = distp.tile([P, K], f32)
                nc.scalar.activation(dist_sb, neg,
                                     mybir.ActivationFunctionType.Identity,
                                     bias=x_sq, scale=-1.0)
                nc.sync.dma_start(out=dist_dram[ts(t, P), :], in_=dist_sb)

                # preds gather: codebook[label] rows
                preds_t = predsp.tile([P, D], f32)
                if NO_GATHER:
                    nc.gpsimd.memset(preds_t, 0.0)
                else:
                    nc.gpsimd.indirect_dma_start(
                        out=preds_t, out_offset=None, in_=cb_dram,
                        in_offset=bass.IndirectOffsetOnAxis(ap=idx8[:, 0:1], axis=0),
                    )
                nc.sync.dma_start(out=preds_dram[ts(t, P), :], in_=preds_t)

                if not NO_LABELS:
                    nc.sync.dma_start(out=labels2d[t, :],
                                      in_=idx8[:, 0:1].bitcast(i32))


    nc.compile()
    return nc


_NC_CACHE = {}


def kernel(x: np.ndarray, codebook: np.ndarray):
    from concourse import bass_utils

    n_tiles = N_TILES_FULL
    if n_tiles not in _NC_CACHE:
        _NC_CACHE[n_tiles] = build(n_tiles)
    nc = _NC_CACHE[n_tiles]

    x = np.ascontiguousarray(x, dtype=np.float32)
    codebook = np.ascontiguousarray(codebook, dtype=np.float32)
    in_maps = [
        {"x": x[i * N_LOCAL:(i + 1) * N_LOCAL], "codebook": codebook}
        for i in range(N_CORES)
    ]
    res = bass_utils.run_bass_kernel_spmd(nc, in_maps, core_ids=list(range(N_CORES)))
    outs = res.results
    preds = np.concatenate([r["preds"] for r in outs], axis=0)
    labels = np.concatenate([r["labels"] for r in outs], axis=0).astype(np.int32)
    dist = np.concatenate([r["dist"] for r in outs], axis=0)
    return preds, labels, dist
